# revision 7
# baseline (speedup 1.0000x reference)
# Trainium2 Bass kernel for NormalAttention (1x1-conv q/k/v attention over HW).
#
# Math (per batch b):
#   q = Wq x + bq            [64, 4096]
#   k = Wk x + bk            [64, 4096]
#   v = Wv x + bv            [256, 4096]
#   E[i,j] = sum_c q[c,i] k[c,j]          (energy, [4096, 4096])
#   A = elu(E) / 4096
#   out = v @ A                           [256, 4096]
#   y = Wg out + bg
#
# Kernel strategy: data-parallel, one batch per NeuronCore (8 cores).
# Per core, a flash-attention-style stream over the energy matrix:
# never materialize E in HBM; for each m-tile of 512 columns, loop over
# pairs of 128-row n-chunks, computing E tiles into PSUM, applying
#   G = min(exp(E), 1) + relu(E) = elu(E) + 1
# and accumulating out' = (v/4096) @ G into PSUM.  The "+1" offset is
# exactly corrected at the end through the gamma bias:
#   out' = out_attn + S 1^T   where S = rowsum(v/4096)
#   y = Wg out' + (bg - Wg S)
# The elementwise elu work is split across ACT / DVE / GPSIMD with
# tunable patterns so no single engine becomes the bottleneck.
import numpy as np
import ml_dtypes

import concourse.bass as bass
import concourse.mybir as mybir
import concourse.tile as tile
from concourse import bacc
from concourse.bass_utils import run_bass_kernel_spmd

B, C, HH, WW = 8, 256, 64, 64
N = HH * WW          # 4096 spatial positions
CQ = 64              # query/key channels
NCORES = 8
MT = 512             # m (energy column) tile
NPAIRS = 16          # pairs of 128-row n-chunks per m-tile
ACT_RELU_EVERY = 3   # 1/ACT_RELU_EVERY of relu passes go to ACT (rest DVE)
DVE_COMBINE_UPTO = 9  # of every 16 combine ops, this many on DVE (rest Pool)

F32 = mybir.dt.float32
F32R = mybir.dt.float32r
BF16 = mybir.dt.bfloat16
AL = mybir.AluOpType
AF = mybir.ActivationFunctionType


def build_nc(reps=1):
    nc = bacc.Bacc("TRN2", target_bir_lowering=False, debug=False,
                   num_devices=NCORES)
    xd = nc.declare_dram_parameter("x", [2, 128, N], F32R, isOutput=False)
    wqd = nc.declare_dram_parameter("wqT", [2, 128, CQ], F32R, isOutput=False)
    wkd = nc.declare_dram_parameter("wkT", [2, 128, CQ], F32R, isOutput=False)
    bqd = nc.declare_dram_parameter("bq", [CQ, 1], F32, isOutput=False)
    bkd = nc.declare_dram_parameter("bk", [CQ, 1], F32, isOutput=False)
    wvd = nc.declare_dram_parameter("wvT", [2, 128, C], F32R, isOutput=False)
    bvd = nc.declare_dram_parameter("bv", [1, C], F32R, isOutput=False)
    wgd = nc.declare_dram_parameter("wgT16", [2, 128, C], BF16, isOutput=False)
    bgd = nc.declare_dram_parameter("bg", [C, 1], F32, isOutput=False)
    onesd = nc.declare_dram_parameter("ones", [1, 128], F32R, isOutput=False)
    od = nc.declare_dram_parameter("out", [2, 128, N], F32, isOutput=True)

    with tile.TileContext(nc) as tc:
        with (
            tc.tile_pool(name="wts", bufs=1) as wts,
            tc.tile_pool(name="xs", bufs=1) as xs,
            tc.tile_pool(name="qk", bufs=1) as qkp,
            tc.tile_pool(name="vt", bufs=1) as vtp,
            tc.tile_pool(name="elem", bufs=3) as elem,
            tc.tile_pool(name="finp", bufs=2) as finp,
            tc.tile_pool(name="pse", bufs=2, space="PSUM") as pse,
            tc.tile_pool(name="pso", bufs=1, space="PSUM") as pso,
            tc.tile_pool(name="psg", bufs=2, space="PSUM") as psg,
        ):
            def body(iv=None):
                x_sb = [xs.tile([128, N], F32R, tag=f"x{i}", name=f"x_sb{i}")
                        for i in range(2)]
                for i in range(2):
                    nc.sync.dma_start(x_sb[i], xd[i])
                wq_sb = wts.tile([128, 2, CQ], F32R, tag="wq", name="wq_sb")
                wk_sb = wts.tile([128, 2, CQ], F32R, tag="wk", name="wk_sb")
                wv_sb = wts.tile([128, 2, C], F32R, tag="wv", name="wv_sb")
                wg_sb = wts.tile([128, 2, C], BF16, tag="wg", name="wg_sb")
                for i in range(2):
                    nc.sync.dma_start(wq_sb[:, i, :], wqd[i])
                    nc.sync.dma_start(wk_sb[:, i, :], wkd[i])
                    nc.sync.dma_start(wv_sb[:, i, :], wvd[i])
                    nc.sync.dma_start(wg_sb[:, i, :], wgd[i])
                bq_sb = wts.tile([CQ, 1], F32, tag="bq", name="bq_sb")
                nc.sync.dma_start(bq_sb, bqd[:])
                bk_sb = wts.tile([CQ, 1], F32, tag="bk", name="bk_sb")
                nc.sync.dma_start(bk_sb, bkd[:])
                bv_sb = wts.tile([1, C], F32R, tag="bv", name="bv_sb")
                nc.sync.dma_start(bv_sb, bvd[:])
                bg_sb = wts.tile([128, 2], F32, tag="bg", name="bg_sb")
                for h in range(2):
                    nc.sync.dma_start(bg_sb[:, h:h + 1],
                                      bgd[h * 128:(h + 1) * 128, :])
                ones_row = wts.tile([1, 128], F32R, tag="ones_row",
                                    name="ones_row")
                nc.sync.dma_start(ones_row, onesd[:])
                ones_col = wts.tile([128, 1], BF16, tag="ones_col",
                                    name="ones_col")
                nc.vector.memset(ones_col, 1.0)

                q_sb = qkp.tile([CQ, N], F32R, tag="q", name="q_sb")
                k_sb = qkp.tile([CQ, N], F32R, tag="k", name="k_sb")
                vt_sb = vtp.tile([128, 32, C], BF16, tag="vt", name="vt_sb")
                sT_sb = wts.tile([1, C], F32, tag="sT", name="sT_sb")
                s_col = wts.tile([128, 2], BF16, tag="scol", name="s_col")
                bge_sb = wts.tile([128, 2], F32, tag="bge", name="bge_sb")

                # ---- q, k = conv1x1(x) + bias   [64, 4096] ----
                for ti in range(N // 512):
                    sl = slice(ti * 512, (ti + 1) * 512)
                    for dst, w_s, b_s in ((q_sb, wq_sb, bq_sb),
                                          (k_sb, wk_sb, bk_sb)):
                        ps = psg.tile([CQ, 512], F32, tag="gps", name="qkps")
                        nc.tensor.matmul(ps, w_s[:, 0, :], x_sb[0][:, sl],
                                         start=True, stop=False)
                        nc.tensor.matmul(ps, w_s[:, 1, :], x_sb[1][:, sl],
                                         start=False, stop=True)
                        nc.scalar.activation(dst[:, sl], ps, AF.Identity,
                                             bias=b_s, scale=1.0)
                # ---- v^T = (x^T WvT + bv)/4096, stored bf16 [4096, 256] ----
                for ni in range(32):
                    nsl = slice(ni * 128, (ni + 1) * 128)
                    ps = pse.tile([128, C], F32, tag="eps", name="vps")
                    nc.tensor.matmul(ps, x_sb[0][:, nsl], wv_sb[:, 0, :],
                                     start=True, stop=False)
                    nc.tensor.matmul(ps, x_sb[1][:, nsl], wv_sb[:, 1, :],
                                     start=False, stop=False)
                    nc.tensor.matmul(ps, ones_row, bv_sb,
                                     start=False, stop=True)
                    nc.vector.tensor_copy(vt_sb[:, ni, :], ps)
                # ---- S = rowsum(v/4096); bg_eff = bg - Wg S ----
                sps = pso.tile([1, C], F32, tag="o0", name="sps")
                for ni in range(32):
                    nc.tensor.matmul(sps, ones_col, vt_sb[:, ni, :],
                                     start=(ni == 0), stop=(ni == 31))
                nc.vector.tensor_copy(sT_sb, sps)
                for h in range(2):
                    # [1,128] row -> [128,1] column (with f32->bf16 cast)
                    nc.gpsimd.dma_start(s_col[:, h:h + 1],
                                        sT_sb[:, h * 128:(h + 1) * 128])
                for h in range(2):
                    hsl = slice(h * 128, (h + 1) * 128)
                    ps = pso.tile([128, 1], F32, tag="o1", name="bgps")
                    nc.tensor.matmul(ps, wg_sb[:, 0, hsl], s_col[:, 0:1],
                                     start=True, stop=False)
                    nc.tensor.matmul(ps, wg_sb[:, 1, hsl], s_col[:, 1:2],
                                     start=False, stop=True)
                    nc.scalar.activation(bge_sb[:, h:h + 1], ps, AF.Identity,
                                         bias=bg_sb[:, h:h + 1], scale=-1.0)

                # ---- main attention loop ----
                for mt in range(N // MT):
                    msl = slice(mt * MT, (mt + 1) * MT)
                    o_ps = [pso.tile([128, MT], F32, tag=f"o{ci}",
                                     name=f"o_ps{ci}") for ci in range(2)]
                    for p in range(NPAIRS):
                        nA, nB = 2 * p, 2 * p + 1
                        eps = pse.tile([128, 2 * MT], F32, tag="eps",
                                       name="eps")
                        nc.tensor.matmul(eps[:, 0:MT],
                                         q_sb[:, nA * 128:(nA + 1) * 128],
                                         k_sb[:, msl], start=True, stop=True)
                        nc.tensor.matmul(eps[:, MT:2 * MT],
                                         q_sb[:, nB * 128:(nB + 1) * 128],
                                         k_sb[:, msl], start=True, stop=True)
                        t16 = elem.tile([128, 2 * MT], BF16, tag="t",
                                        name="t16")
                        a16 = elem.tile([128, 2 * MT], BF16, tag="a",
                                        name="a16")
                        g16 = elem.tile([128, 2 * MT], BF16, tag="g",
                                        name="g16")
                        # G = min(exp(E),1) + relu(E) = elu(E)+1 exactly.
                        nc.scalar.activation(t16, eps, AF.Exp)
                        if p % ACT_RELU_EVERY == 1:
                            nc.scalar.activation(a16, eps, AF.Relu)
                        else:
                            nc.vector.tensor_scalar(a16, eps, 0.0, None,
                                                    AL.max)
                        if p % 16 < DVE_COMBINE_UPTO:
                            nc.vector.scalar_tensor_tensor(
                                g16, t16, 1.0, a16, AL.min, AL.add)
                        else:
                            u16 = elem.tile([128, 2 * MT], BF16, tag="u",
                                            name="u16")
                            nc.gpsimd.tensor_scalar(u16, t16, 1.0, None,
                                                    AL.min)
                            nc.gpsimd.tensor_tensor(g16, u16, a16, AL.add)
                        for ci in range(2):
                            csl = slice(ci * 128, (ci + 1) * 128)
                            nc.tensor.matmul(o_ps[ci], vt_sb[:, nA, csl],
                                             g16[:, 0:MT],
                                             start=(p == 0), stop=False)
                            nc.tensor.matmul(o_ps[ci], vt_sb[:, nB, csl],
                                             g16[:, MT:2 * MT],
                                             start=False,
                                             stop=(p == NPAIRS - 1))
                    osb = []
                    for ci in range(2):
                        ob = finp.tile([128, MT], BF16, tag=f"ob{ci}",
                                       name=f"ob{ci}")
                        nc.vector.tensor_copy(ob, o_ps[ci])
                        osb.append(ob)
                    for h in range(2):
                        hsl = slice(h * 128, (h + 1) * 128)
                        gps = psg.tile([128, MT], F32, tag="gps", name="gps")
                        nc.tensor.matmul(gps, wg_sb[:, 0, hsl], osb[0],
                                         start=True, stop=False)
                        nc.tensor.matmul(gps, wg_sb[:, 1, hsl], osb[1],
                                         start=False, stop=True)
                        fo = finp.tile([128, MT], F32, tag="fo", name="fo")
                        nc.vector.tensor_scalar(fo, gps, bge_sb[:, h:h + 1],
                                                None, AL.add)
                        nc.sync.dma_start(od[h, :, msl], fo)

            if reps == 1:
                body()
            else:
                with tc.For_i(0, reps, 1):
                    body()
    nc.compile()
    return nc


_NC_CACHE = {}


def _get_nc(reps=1):
    if reps not in _NC_CACHE:
        _NC_CACHE[reps] = build_nc(reps)
    return _NC_CACHE[reps]


def _prep_in_maps(inputs):
    x = np.ascontiguousarray(np.asarray(inputs["x"], dtype=np.float32))
    wq = np.asarray(inputs["query_weight"], np.float32)[:, :, 0, 0]
    bq = np.asarray(inputs["query_bias"], np.float32)
    wk = np.asarray(inputs["key_weight"], np.float32)[:, :, 0, 0]
    bk = np.asarray(inputs["key_bias"], np.float32)
    wv = np.asarray(inputs["value_weight"], np.float32)[:, :, 0, 0]
    bv = np.asarray(inputs["value_bias"], np.float32)
    wg = np.asarray(inputs["gamma_weight"], np.float32)[:, :, 0, 0]
    bg = np.asarray(inputs["gamma_bias"], np.float32)

    wqT = np.ascontiguousarray(wq.T).reshape(2, 128, CQ)
    wkT = np.ascontiguousarray(wk.T).reshape(2, 128, CQ)
    wvT = np.ascontiguousarray(wv.T / N).reshape(2, 128, C).astype(np.float32)
    bvr = (bv / N).reshape(1, C).astype(np.float32)
    wgT16 = np.ascontiguousarray(wg.T).astype(ml_dtypes.bfloat16).reshape(
        2, 128, C)
    shared = {
        "wqT": wqT, "wkT": wkT,
        "bq": np.ascontiguousarray(bq.reshape(CQ, 1)),
        "bk": np.ascontiguousarray(bk.reshape(CQ, 1)),
        "wvT": wvT, "bv": bvr, "wgT16": wgT16,
        "bg": np.ascontiguousarray(bg.reshape(C, 1)),
        "ones": np.ones((1, 128), np.float32),
    }
    return [dict(shared, x=x[b].reshape(2, 128, N)) for b in range(B)]


def _run(inputs, trace=False, reps=1):
    nc = _get_nc(reps)
    in_maps = _prep_in_maps(inputs)
    res = run_bass_kernel_spmd(nc, in_maps, core_ids=list(range(NCORES)),
                               trace=trace)
    out = np.stack([r["out"].reshape(C, HH, WW) for r in res.results], axis=0)
    return out, res


def kernel(**inputs):
    out, _ = _run(inputs, trace=False)
    return out


# revision 8
# speedup vs baseline: 1.4845x; 1.4845x over previous
# Trainium2 Bass kernel for NormalAttention (1x1-conv q/k/v attention over HW).
#
# Math (per batch b):
#   q = Wq x + bq            [64, 4096]
#   k = Wk x + bk            [64, 4096]
#   v = Wv x + bv            [256, 4096]
#   E[i,j] = sum_c q[c,i] k[c,j]          (energy, [4096, 4096])
#   A = elu(E) / 4096
#   out = v @ A                           [256, 4096]
#   y = Wg out + bg
#
# Kernel strategy: data-parallel, one batch per NeuronCore (8 cores).
# Per core, a flash-attention-style stream over the energy matrix:
# never materialize E in HBM; for each m-tile of 512 columns, loop over
# pairs of 128-row n-chunks, computing E tiles into PSUM, applying
#   G = min(exp(E), 1) + relu(E) = elu(E) + 1
# and accumulating out' = (v/4096) @ G into PSUM.  The "+1" offset is
# exactly corrected at the end through the gamma bias:
#   out' = out_attn + S 1^T   where S = rowsum(v/4096)
#   y = Wg out' + (bg - Wg S)
# The elementwise elu work is split across ACT / DVE / GPSIMD with
# tunable patterns so no single engine becomes the bottleneck.
import numpy as np
import ml_dtypes

import concourse.bass as bass
import concourse.mybir as mybir
import concourse.tile as tile
from concourse import bacc
from concourse.bass_utils import run_bass_kernel_spmd

B, C, HH, WW = 8, 256, 64, 64
N = HH * WW          # 4096 spatial positions
CQ = 64              # query/key channels
NCORES = 8
MT = 512             # m (energy column) tile
NPAIRS = 16          # pairs of 128-row n-chunks per m-tile
ACT_RELU_EVERY = 3   # 1/ACT_RELU_EVERY of relu passes go to ACT (rest DVE)
DVE_COMBINE_UPTO = 9  # of every 16 combine ops, this many on DVE (rest Pool)

F32 = mybir.dt.float32
F32R = mybir.dt.float32r
BF16 = mybir.dt.bfloat16
AL = mybir.AluOpType
AF = mybir.ActivationFunctionType


def build_nc(reps=1):
    nc = bacc.Bacc("TRN2", target_bir_lowering=False, debug=False,
                   num_devices=NCORES)
    xd = nc.declare_dram_parameter("x", [2, 128, N], F32R, isOutput=False)
    wqd = nc.declare_dram_parameter("wqT", [2, 128, CQ], F32R, isOutput=False)
    wkd = nc.declare_dram_parameter("wkT", [2, 128, CQ], F32R, isOutput=False)
    bqd = nc.declare_dram_parameter("bq", [CQ, 1], F32, isOutput=False)
    bkd = nc.declare_dram_parameter("bk", [CQ, 1], F32, isOutput=False)
    wvd = nc.declare_dram_parameter("wvT", [2, 128, C], F32R, isOutput=False)
    bvd = nc.declare_dram_parameter("bv", [1, C], F32R, isOutput=False)
    wgd = nc.declare_dram_parameter("wgT16", [2, 128, C], BF16, isOutput=False)
    bgd = nc.declare_dram_parameter("bg", [C, 1], F32, isOutput=False)
    onesd = nc.declare_dram_parameter("ones", [1, 128], F32R, isOutput=False)
    od = nc.declare_dram_parameter("out", [2, 128, N], F32, isOutput=True)

    with tile.TileContext(nc) as tc:
        with (
            tc.tile_pool(name="wts", bufs=1) as wts,
            tc.tile_pool(name="xs", bufs=1) as xs,
            tc.tile_pool(name="qk", bufs=1) as qkp,
            tc.tile_pool(name="vt", bufs=1) as vtp,
            tc.tile_pool(name="elem", bufs=3) as elem,
            tc.tile_pool(name="finp", bufs=2) as finp,
            tc.tile_pool(name="pse", bufs=2, space="PSUM") as pse,
            tc.tile_pool(name="pso", bufs=1, space="PSUM") as pso,
            tc.tile_pool(name="psg", bufs=2, space="PSUM") as psg,
        ):
            def body(iv=None):
                x_sb = [xs.tile([128, N], F32R, tag=f"x{i}", name=f"x_sb{i}")
                        for i in range(2)]
                for i in range(2):
                    for cch in range(4):
                        cs = slice(cch * (N // 4), (cch + 1) * (N // 4))
                        nc.sync.dma_start(x_sb[i][:, cs], xd[i][:, cs])
                wq_sb = wts.tile([128, 2, CQ], F32R, tag="wq", name="wq_sb")
                wk_sb = wts.tile([128, 2, CQ], F32R, tag="wk", name="wk_sb")
                wv_sb = wts.tile([128, 2, C], F32R, tag="wv", name="wv_sb")
                wg_sb = wts.tile([128, 2, C], BF16, tag="wg", name="wg_sb")
                for i in range(2):
                    nc.sync.dma_start(wq_sb[:, i, :], wqd[i])
                    nc.sync.dma_start(wk_sb[:, i, :], wkd[i])
                    nc.sync.dma_start(wv_sb[:, i, :], wvd[i])
                    nc.sync.dma_start(wg_sb[:, i, :], wgd[i])
                bq_sb = wts.tile([CQ, 1], F32, tag="bq", name="bq_sb")
                nc.sync.dma_start(bq_sb, bqd[:])
                bk_sb = wts.tile([CQ, 1], F32, tag="bk", name="bk_sb")
                nc.sync.dma_start(bk_sb, bkd[:])
                bv_sb = wts.tile([1, C], F32R, tag="bv", name="bv_sb")
                nc.sync.dma_start(bv_sb, bvd[:])
                bg_sb = wts.tile([128, 2], F32, tag="bg", name="bg_sb")
                for h in range(2):
                    nc.sync.dma_start(bg_sb[:, h:h + 1],
                                      bgd[h * 128:(h + 1) * 128, :])
                ones_row = wts.tile([1, 128], F32R, tag="ones_row",
                                    name="ones_row")
                nc.sync.dma_start(ones_row, onesd[:])
                ones_col = wts.tile([128, 1], BF16, tag="ones_col",
                                    name="ones_col")
                nc.vector.memset(ones_col, 1.0)

                q_sb = qkp.tile([2 * CQ, N], F32R, tag="q", name="q_sb")
                k_sb = qkp.tile([2 * CQ, N], F32R, tag="k", name="k_sb")
                vt_sb = vtp.tile([128, 32, C], BF16, tag="vt", name="vt_sb")
                sT_sb = wts.tile([1, C], F32, tag="sT", name="sT_sb")
                s_col = wts.tile([128, 2], BF16, tag="scol", name="s_col")
                bge_sb = wts.tile([128, 2], F32, tag="bge", name="bge_sb")

                # ---- q, k = conv1x1(x) + bias   [64, 4096] ----
                for ti in range(N // 512):
                    sl = slice(ti * 512, (ti + 1) * 512)
                    for dst, w_s, b_s in ((q_sb, wq_sb, bq_sb),
                                          (k_sb, wk_sb, bk_sb)):
                        ps = psg.tile([CQ, 512], F32, tag="gps", name="qkps")
                        nc.tensor.matmul(ps, w_s[:, 0, :], x_sb[0][:, sl],
                                         start=True, stop=False)
                        nc.tensor.matmul(ps, w_s[:, 1, :], x_sb[1][:, sl],
                                         start=False, stop=True)
                        nc.scalar.activation(dst[:CQ, sl], ps, AF.Identity,
                                             bias=b_s, scale=1.0)
                # duplicate q/k into partitions 64..127 (PE row-group packing)
                for dst in (q_sb, k_sb):
                    nc.sync.dma_start(dst[CQ:2 * CQ, :], dst[:CQ, :])
                # ---- v^T = (x^T WvT + bv)/4096, stored bf16 [4096, 256] ----
                for ni in range(32):
                    nsl = slice(ni * 128, (ni + 1) * 128)
                    ps = pse.tile([128, C], F32, tag="eps", name="vps")
                    nc.tensor.matmul(ps, x_sb[0][:, nsl], wv_sb[:, 0, :],
                                     start=True, stop=False)
                    nc.tensor.matmul(ps, x_sb[1][:, nsl], wv_sb[:, 1, :],
                                     start=False, stop=False)
                    nc.tensor.matmul(ps, ones_row, bv_sb,
                                     start=False, stop=True)
                    nc.vector.tensor_copy(vt_sb[:, ni, :], ps)
                # ---- S = rowsum(v/4096); bg_eff = bg - Wg S ----
                sps = pso.tile([1, C], F32, tag="o0", name="sps")
                for ni in range(32):
                    nc.tensor.matmul(sps, ones_col, vt_sb[:, ni, :],
                                     start=(ni == 0), stop=(ni == 31))
                nc.vector.tensor_copy(sT_sb, sps)
                for h in range(2):
                    # [1,128] row -> [128,1] column (with f32->bf16 cast)
                    nc.gpsimd.dma_start(s_col[:, h:h + 1],
                                        sT_sb[:, h * 128:(h + 1) * 128])
                for h in range(2):
                    hsl = slice(h * 128, (h + 1) * 128)
                    ps = pso.tile([128, 1], F32, tag="o1", name="bgps")
                    nc.tensor.matmul(ps, wg_sb[:, 0, hsl], s_col[:, 0:1],
                                     start=True, stop=False)
                    nc.tensor.matmul(ps, wg_sb[:, 1, hsl], s_col[:, 1:2],
                                     start=False, stop=True)
                    nc.scalar.activation(bge_sb[:, h:h + 1], ps, AF.Identity,
                                         bias=bg_sb[:, h:h + 1], scale=-1.0)

                # ---- main attention loop ----
                for mt in range(N // MT):
                    msl = slice(mt * MT, (mt + 1) * MT)
                    o_ps = [pso.tile([128, MT], F32, tag=f"o{ci}",
                                     name=f"o_ps{ci}") for ci in range(2)]
                    for p in range(NPAIRS):
                        nA, nB = 2 * p, 2 * p + 1
                        eps = pse.tile([128, 2 * MT], F32, tag="eps",
                                       name="eps")
                        nc.tensor.matmul(eps[:, 0:MT],
                                         q_sb[:CQ, nA * 128:(nA + 1) * 128],
                                         k_sb[:CQ, msl],
                                         start=True, stop=True)
                        nc.tensor.matmul(eps[:, MT:2 * MT],
                                         q_sb[CQ:2 * CQ,
                                              nB * 128:(nB + 1) * 128],
                                         k_sb[CQ:2 * CQ, msl],
                                         start=True, stop=True)
                        t16 = elem.tile([128, 2 * MT], BF16, tag="t",
                                        name="t16")
                        a16 = elem.tile([128, 2 * MT], BF16, tag="a",
                                        name="a16")
                        g16 = elem.tile([128, 2 * MT], BF16, tag="g",
                                        name="g16")
                        # G = min(exp(E),1) + relu(E) = elu(E)+1 exactly.
                        nc.scalar.activation(t16, eps, AF.Exp)
                        if p % ACT_RELU_EVERY == 1:
                            nc.scalar.activation(a16, eps, AF.Relu)
                        else:
                            nc.vector.tensor_scalar(a16, eps, 0.0, None,
                                                    AL.max)
                        if p % 16 < DVE_COMBINE_UPTO:
                            nc.vector.scalar_tensor_tensor(
                                g16, t16, 1.0, a16, AL.min, AL.add)
                        else:
                            u16 = elem.tile([128, 2 * MT], BF16, tag="u",
                                            name="u16")
                            nc.gpsimd.tensor_scalar(u16, t16, 1.0, None,
                                                    AL.min)
                            nc.gpsimd.tensor_tensor(g16, u16, a16, AL.add)
                        for ci in range(2):
                            csl = slice(ci * 128, (ci + 1) * 128)
                            nc.tensor.matmul(o_ps[ci], vt_sb[:, nA, csl],
                                             g16[:, 0:MT],
                                             start=(p == 0), stop=False)
                            nc.tensor.matmul(o_ps[ci], vt_sb[:, nB, csl],
                                             g16[:, MT:2 * MT],
                                             start=False,
                                             stop=(p == NPAIRS - 1))
                    osb = []
                    for ci in range(2):
                        ob = finp.tile([128, MT], BF16, tag=f"ob{ci}",
                                       name=f"ob{ci}")
                        nc.vector.tensor_copy(ob, o_ps[ci])
                        osb.append(ob)
                    for h in range(2):
                        hsl = slice(h * 128, (h + 1) * 128)
                        gps = psg.tile([128, MT], F32, tag="gps", name="gps")
                        nc.tensor.matmul(gps, wg_sb[:, 0, hsl], osb[0],
                                         start=True, stop=False)
                        nc.tensor.matmul(gps, wg_sb[:, 1, hsl], osb[1],
                                         start=False, stop=True)
                        fo = finp.tile([128, MT], F32, tag="fo", name="fo")
                        nc.vector.tensor_scalar(fo, gps, bge_sb[:, h:h + 1],
                                                None, AL.add)
                        nc.sync.dma_start(od[h, :, msl], fo)

            if reps == 1:
                body()
            else:
                with tc.For_i(0, reps, 1):
                    body()
    nc.compile()
    return nc


_NC_CACHE = {}


def _get_nc(reps=1):
    if reps not in _NC_CACHE:
        _NC_CACHE[reps] = build_nc(reps)
    return _NC_CACHE[reps]


def _prep_in_maps(inputs):
    x = np.ascontiguousarray(np.asarray(inputs["x"], dtype=np.float32))
    wq = np.asarray(inputs["query_weight"], np.float32)[:, :, 0, 0]
    bq = np.asarray(inputs["query_bias"], np.float32)
    wk = np.asarray(inputs["key_weight"], np.float32)[:, :, 0, 0]
    bk = np.asarray(inputs["key_bias"], np.float32)
    wv = np.asarray(inputs["value_weight"], np.float32)[:, :, 0, 0]
    bv = np.asarray(inputs["value_bias"], np.float32)
    wg = np.asarray(inputs["gamma_weight"], np.float32)[:, :, 0, 0]
    bg = np.asarray(inputs["gamma_bias"], np.float32)

    wqT = np.ascontiguousarray(wq.T).reshape(2, 128, CQ)
    wkT = np.ascontiguousarray(wk.T).reshape(2, 128, CQ)
    wvT = np.ascontiguousarray(wv.T / N).reshape(2, 128, C).astype(np.float32)
    bvr = (bv / N).reshape(1, C).astype(np.float32)
    wgT16 = np.ascontiguousarray(wg.T).astype(ml_dtypes.bfloat16).reshape(
        2, 128, C)
    shared = {
        "wqT": wqT, "wkT": wkT,
        "bq": np.ascontiguousarray(bq.reshape(CQ, 1)),
        "bk": np.ascontiguousarray(bk.reshape(CQ, 1)),
        "wvT": wvT, "bv": bvr, "wgT16": wgT16,
        "bg": np.ascontiguousarray(bg.reshape(C, 1)),
        "ones": np.ones((1, 128), np.float32),
    }
    return [dict(shared, x=x[b].reshape(2, 128, N)) for b in range(B)]


def _run(inputs, trace=False, reps=1):
    nc = _get_nc(reps)
    in_maps = _prep_in_maps(inputs)
    res = run_bass_kernel_spmd(nc, in_maps, core_ids=list(range(NCORES)),
                               trace=trace)
    out = np.stack([r["out"].reshape(C, HH, WW) for r in res.results], axis=0)
    return out, res


def kernel(**inputs):
    out, _ = _run(inputs, trace=False)
    return out
